# revision 20
# baseline (speedup 1.0000x reference)
"""Trainium2 distributed kernel for the AppearanceReconstruction loss.

Math note (exact identity, not an approximation): the MAE shuffle/gather in
the reference collapses — restored[b,p] is appearance_tokens[b,p] on kept
slots (which the mask multiplies by zero) and mask_token on masked slots.
Every row has exactly num_masked = 2 masked slots, and the decoder output at
a masked slot is the same single vector r = MLP(LN(mask_token)) for all
(b,p). Hence

    loss = 2 * sum_b mean_c((r_c - pooled[b,c])^2) / (256 + 1e-8)
    pooled[b] = mean_n target_features[b,n,:]

The memory-bound part (reading all of target_features, 402 MB) runs on the 8
NeuronCores, data-parallel over B (16 rows per core). Each core streams its
50 MB shard through SBUF in 3 MB tiles and reduces over N with TensorEngine
matmuls against one-hot columns (scaled by 1/N), accumulating the 16 row
means into a [16, 768] PSUM tile. A short vector-engine epilogue computes
sum_c (pooled - r)^2 per row; the host sums the 8x16 partials.
"""

import math

import numpy as np

B, N, C = 128, 1024, 768
NCORES = 8
BPC = B // NCORES  # rows per core
PPB = 128  # SBUF partitions per row-tile
NSUB = N // PPB  # n-rows folded into each partition's free dim
FREE = NSUB * C  # 6144 floats per partition per row-tile
LN_EPS = 1e-5

_CACHE = {}

# kernel structure knobs (A/B-tested on hardware; defaults = best measured)
_VARIANT = {"last_split": True, "out_ring": "scalar", "ring_alt": False}


def _build():
    import concourse.bass as bass  # noqa: F401
    import concourse.tile as tile
    from concourse import bacc, mybir

    f32 = mybir.dt.float32
    f32r = mybir.dt.float32r
    AL = mybir.AluOpType
    AX = mybir.AxisListType

    nc = bacc.Bacc(
        "TRN2", target_bir_lowering=False, debug=False, num_devices=NCORES
    )
    tf = nc.dram_tensor("tf", [BPC, PPB, FREE], f32r, kind="ExternalInput")
    rrep = nc.dram_tensor("rrep", [BPC, C], f32, kind="ExternalInput")
    emat = nc.dram_tensor("emat", [PPB, BPC * BPC], f32r, kind="ExternalInput")
    out = nc.dram_tensor("out", [BPC, 2], f32, kind="ExternalOutput")

    with tile.TileContext(nc) as tc:
        with (
            tc.tile_pool(name="consts", bufs=1) as cpool,
            tc.tile_pool(name="data", bufs=6) as dpool,
            tc.tile_pool(name="epi", bufs=1) as epool,
            tc.tile_pool(name="psum", bufs=1, space="PSUM") as ppool,
        ):
            # issue the first data tile's DMA before the tiny const loads so
            # the 3 MB stream starts as early as possible
            t0_tile = dpool.tile([PPB, FREE], f32r, tag="data")
            nc.sync.dma_start(out=t0_tile[:], in_=tf.ap()[0])

            # const loads go on the ACT HWDGE ring so the SP ring's first job
            # is the 3 MB stream itself
            emat_sb = cpool.tile([PPB, BPC * BPC], f32r)
            nc.scalar.dma_start(out=emat_sb[:], in_=emat.ap())
            rrep_sb = cpool.tile([BPC, C], f32)
            nc.scalar.dma_start(out=rrep_sb[:], in_=rrep.ap())

            psA = ppool.tile([BPC, 512], f32)
            psB = ppool.tile([BPC, 256], f32)

            # (row, sub_lo, sub_hi) chunks; full 3 MB rows keep the DMA
            # stream at peak rate, only the last row is halved so the
            # post-final-DMA PE tail is half a row
            half = NSUB // 2
            if _VARIANT["last_split"]:
                chunks = [(b, 0, NSUB) for b in range(BPC - 1)]
                chunks += [(BPC - 1, 0, half), (BPC - 1, half, NSUB)]
            else:
                chunks = [(b, 0, NSUB) for b in range(BPC)]

            for ci, (b, lo, hi) in enumerate(chunks):
                if ci == 0:
                    t = t0_tile
                else:
                    t = dpool.tile([PPB, (hi - lo) * C], f32r, tag="data")
                    # optionally alternate the two HWDGE rings (SP/ACT) so
                    # descriptor generation of consecutive transfers overlaps;
                    # the final two chunks stay on one ring to preserve their
                    # completion order (the PE tail depends on the last chunk
                    # alone finishing last)
                    if _VARIANT["ring_alt"] and ci < len(chunks) - 2:
                        eng = nc.sync if ci % 2 == 0 else nc.scalar
                    else:
                        eng = nc.sync
                    eng.dma_start(
                        out=t[:], in_=tf.ap()[b, :, lo * C : hi * C]
                    )
                # float32r: same 4-byte layout, 4x faster PE streaming; the
                # reduced-precision multiply is far inside the loss tolerance.
                lhsT = emat_sb[:, b * BPC : (b + 1) * BPC]
                first = ci == 0
                last = ci == len(chunks) - 1
                for sub in range(lo, hi):
                    nc.tensor.matmul(
                        psA[:],
                        lhsT,
                        t[:, (sub - lo) * C : (sub - lo) * C + 512],
                        start=first and sub == lo,
                        stop=last and sub == hi - 1,
                    )
                for sub in range(lo, hi):
                    nc.tensor.matmul(
                        psB[:],
                        lhsT,
                        t[:, (sub - lo) * C + 512 : (sub - lo + 1) * C],
                        start=first and sub == lo,
                        stop=last and sub == hi - 1,
                    )

            # two independent chains (psA half / psB half) so the A chain
            # starts as soon as psA's accumulation stops, overlapping the
            # last psB matmuls
            d = epool.tile([BPC, C], f32)
            sq = epool.tile([BPC, C], f32)
            s = epool.tile([BPC, 2], f32)
            nc.vector.tensor_tensor(
                out=d[:, 0:512], in0=psA[:], in1=rrep_sb[:, 0:512], op=AL.subtract
            )
            nc.vector.tensor_tensor(
                out=sq[:, 0:512], in0=d[:, 0:512], in1=d[:, 0:512], op=AL.mult
            )
            nc.vector.tensor_reduce(
                out=s[:, 0:1], in_=sq[:, 0:512], axis=AX.X, op=AL.add
            )
            nc.vector.tensor_tensor(
                out=d[:, 512:768], in0=psB[:], in1=rrep_sb[:, 512:768], op=AL.subtract
            )
            nc.vector.tensor_tensor(
                out=sq[:, 512:768],
                in0=d[:, 512:768],
                in1=d[:, 512:768],
                op=AL.mult,
            )
            nc.vector.tensor_reduce(
                out=s[:, 1:2], in_=sq[:, 512:768], axis=AX.X, op=AL.add
            )
            # output DMA on the ACT HWDGE ring so it never queues behind the
            # SP ring's bulk data stream
            out_eng = nc.scalar if _VARIANT["out_ring"] == "scalar" else nc.sync
            out_eng.dma_start(out=out.ap(), in_=s[:])

    nc.compile()
    return nc


def _get_nc():
    nc = _CACHE.get("nc")
    if nc is None:
        nc = _build()
        _CACHE["nc"] = nc
    return nc


def _host_r(mask_token, ln_w, ln_b, W1, b1, W2, b2):
    """r = Linear2(gelu_exact(Linear1(LayerNorm(mask_token)))) — one 768-vec."""
    mt = np.asarray(mask_token, np.float64).reshape(C)
    mu = mt.mean()
    var = ((mt - mu) ** 2).mean()
    x = (mt - mu) / np.sqrt(var + LN_EPS) * np.asarray(ln_w, np.float64) + np.asarray(
        ln_b, np.float64
    )
    h = x @ np.asarray(W1, np.float64) + np.asarray(b1, np.float64)
    erf = np.frompyfunc(math.erf, 1, 1)
    g = h * 0.5 * (1.0 + erf(h / math.sqrt(2.0)).astype(np.float64))
    r = g @ np.asarray(W2, np.float64) + np.asarray(b2, np.float64)
    return r.astype(np.float32)


def kernel(
    appearance_tokens,
    target_features,
    noise,
    mask_token,
    ln_w,
    ln_b,
    W1,
    b1,
    W2,
    b2,
):
    from concourse.bass_utils import run_bass_kernel_spmd

    nc = _get_nc()

    r = _host_r(mask_token, ln_w, ln_b, W1, b1, W2, b2)
    rrep = np.ascontiguousarray(np.broadcast_to(r, (BPC, C)), np.float32)

    # emat[:, b*16+m] = 1/N if m == b else 0 — one-hot columns scaled so the
    # partition-reduction matmul lands mean_n directly in PSUM row b.
    emat = np.zeros((PPB, BPC * BPC), np.float32)
    for b in range(BPC):
        emat[:, b * BPC + b] = 1.0 / N

    tfull = np.ascontiguousarray(target_features, np.float32).reshape(
        NCORES, BPC, PPB, FREE
    )
    in_maps = [
        {"tf": tfull[i], "rrep": rrep, "emat": emat} for i in range(NCORES)
    ]

    res = run_bass_kernel_spmd(nc, in_maps, list(range(NCORES)))
    total = 0.0
    for i in range(NCORES):
        total += float(np.asarray(res.results[i]["out"], np.float64).sum())

    loss = 2.0 * total / C / (256.0 + 1e-8)
    return np.float32(loss)


# revision 26
# speedup vs baseline: 1.0712x; 1.0712x over previous
"""Trainium2 distributed kernel for the AppearanceReconstruction loss.

Math note (exact identity, not an approximation): the MAE shuffle/gather in
the reference collapses — restored[b,p] is appearance_tokens[b,p] on kept
slots (which the mask multiplies by zero) and mask_token on masked slots.
Every row has exactly num_masked = 2 masked slots, and the decoder output at
a masked slot is the same single vector r = MLP(LN(mask_token)) for all
(b,p). Hence

    loss = 2 * sum_b mean_c((r_c - pooled[b,c])^2) / (256 + 1e-8)
    pooled[b] = mean_n target_features[b,n,:]

The memory-bound part (reading all of target_features, 402 MB) runs on the 8
NeuronCores, data-parallel over B (16 rows per core). Each core streams its
50 MB shard through SBUF in 3 MB tiles and reduces over N with TensorEngine
matmuls against one-hot columns (scaled by 1/N), accumulating the 16 row
means into a [16, 768] PSUM tile. A short vector-engine epilogue computes
sum_c (pooled - r)^2 per row; the host sums the 8x16 partials.
"""

import math

import numpy as np

B, N, C = 128, 1024, 768
NCORES = 8
BPC = B // NCORES  # rows per core
PPB = 128  # SBUF partitions per row-tile
NSUB = N // PPB  # n-rows folded into each partition's free dim
FREE = NSUB * C  # 6144 floats per partition per row-tile
LN_EPS = 1e-5

_CACHE = {}

# kernel structure knobs (A/B-tested on hardware; defaults = best measured)
_VARIANT = {"last_split": True, "out_ring": "scalar", "ring_alt": False}


def _build():
    import concourse.bass as bass  # noqa: F401
    import concourse.tile as tile
    from concourse import bacc, mybir

    f32 = mybir.dt.float32
    f32r = mybir.dt.float32r
    AL = mybir.AluOpType
    AX = mybir.AxisListType

    nc = bacc.Bacc(
        "TRN2", target_bir_lowering=False, debug=False, num_devices=NCORES
    )
    tf = nc.dram_tensor("tf", [BPC, PPB, FREE], f32r, kind="ExternalInput")
    negr = nc.dram_tensor("negr", [1, C], f32r, kind="ExternalInput")
    ones16 = nc.dram_tensor("ones16", [1, BPC], f32r, kind="ExternalInput")
    emat = nc.dram_tensor("emat", [PPB, BPC * BPC], f32r, kind="ExternalInput")
    out = nc.dram_tensor("out", [BPC, 1], f32, kind="ExternalOutput")

    with tile.TileContext(nc) as tc:
        with (
            tc.tile_pool(name="consts", bufs=1) as cpool,
            tc.tile_pool(name="data", bufs=6) as dpool,
            tc.tile_pool(name="epi", bufs=1) as epool,
            tc.tile_pool(name="psum", bufs=1, space="PSUM") as ppool,
        ):
            # issue the first data tile's DMA before the tiny const loads so
            # the 3 MB stream starts as early as possible
            t0_tile = dpool.tile([PPB, FREE], f32r, tag="data")
            nc.sync.dma_start(out=t0_tile[:], in_=tf.ap()[0])

            # const loads go on the ACT HWDGE ring so the SP ring's first job
            # is the 3 MB stream itself
            emat_sb = cpool.tile([PPB, BPC * BPC], f32r)
            nc.scalar.dma_start(out=emat_sb[:], in_=emat.ap())
            negr_sb = cpool.tile([1, C], f32r)
            nc.scalar.dma_start(out=negr_sb[:], in_=negr.ap())
            ones16_sb = cpool.tile([1, BPC], f32r)
            nc.scalar.dma_start(out=ones16_sb[:], in_=ones16.ap())

            # single [16, 768] accumulator spanning two PSUM banks; each
            # matmul's out AP stays within one bank (512 | 256)
            ps = ppool.tile([BPC, C], f32)

            # (row, sub_lo, sub_hi) chunks; full 3 MB rows keep the DMA
            # stream at peak rate, only the last row is halved so the
            # post-final-DMA PE tail is half a row
            half = NSUB // 2
            quart = NSUB // 4
            if _VARIANT["last_split"]:
                chunks = [(b, 0, NSUB) for b in range(BPC - 1)]
                chunks += [
                    (BPC - 1, 0, half),
                    (BPC - 1, half, half + quart),
                    (BPC - 1, half + quart, NSUB),
                ]
            else:
                chunks = [(b, 0, NSUB) for b in range(BPC)]

            for ci, (b, lo, hi) in enumerate(chunks):
                if ci == 0:
                    t = t0_tile
                else:
                    t = dpool.tile([PPB, (hi - lo) * C], f32r, tag="data")
                    # optionally alternate the two HWDGE rings (SP/ACT) so
                    # descriptor generation of consecutive transfers overlaps;
                    # the final two chunks stay on one ring to preserve their
                    # completion order (the PE tail depends on the last chunk
                    # alone finishing last)
                    if _VARIANT["ring_alt"] and ci < len(chunks) - 2:
                        eng = nc.sync if ci % 2 == 0 else nc.scalar
                    else:
                        eng = nc.sync
                    eng.dma_start(
                        out=t[:], in_=tf.ap()[b, :, lo * C : hi * C]
                    )
                # float32r: same 4-byte layout, 4x faster PE streaming; the
                # reduced-precision multiply is far inside the loss tolerance.
                lhsT = emat_sb[:, b * BPC : (b + 1) * BPC]
                first = ci == 0
                last = ci == len(chunks) - 1
                for sub in range(lo, hi):
                    nc.tensor.matmul(
                        ps[:, 0:512],
                        lhsT,
                        t[:, (sub - lo) * C : (sub - lo) * C + 512],
                        start=first and sub == lo,
                        stop=last and sub == hi - 1,
                    )
                for sub in range(lo, hi):
                    nc.tensor.matmul(
                        ps[:, 512:768],
                        lhsT,
                        t[:, (sub - lo) * C + 512 : (sub - lo + 1) * C],
                        start=first and sub == lo,
                        stop=last and sub == hi - 1,
                    )
                if ci == 0:
                    # fold the "- r" into the accumulation: one K=1 matmul
                    # adds -r_c to every row, early so it is off the tail.
                    # PSUM then holds (pooled_mean - r) directly and the
                    # epilogue shrinks to square + reduce.
                    nc.tensor.matmul(
                        ps[:, 0:512],
                        ones16_sb[:],
                        negr_sb[:, 0:512],
                        start=False,
                        stop=False,
                    )
                    nc.tensor.matmul(
                        ps[:, 512:768],
                        ones16_sb[:],
                        negr_sb[:, 512:768],
                        start=False,
                        stop=False,
                    )

            # one ACT instruction: square every element of (pooled - r) and
            # row-sum into s — single PSUM read, runs on the idle ACT engine
            sq = epool.tile([BPC, C], f32)
            s = epool.tile([BPC, 1], f32)
            nc.scalar.activation(
                out=sq[:],
                in_=ps[:],
                func=mybir.ActivationFunctionType.Square,
                accum_out=s[:],
            )
            # output DMA on the ACT HWDGE ring so it never queues behind the
            # SP ring's bulk data stream
            out_eng = nc.scalar if _VARIANT["out_ring"] == "scalar" else nc.sync
            out_eng.dma_start(out=out.ap(), in_=s[:])

    nc.compile()
    return nc


def _get_nc():
    nc = _CACHE.get("nc")
    if nc is None:
        nc = _build()
        _CACHE["nc"] = nc
    return nc


def _host_r(mask_token, ln_w, ln_b, W1, b1, W2, b2):
    """r = Linear2(gelu_exact(Linear1(LayerNorm(mask_token)))) — one 768-vec."""
    mt = np.asarray(mask_token, np.float64).reshape(C)
    mu = mt.mean()
    var = ((mt - mu) ** 2).mean()
    x = (mt - mu) / np.sqrt(var + LN_EPS) * np.asarray(ln_w, np.float64) + np.asarray(
        ln_b, np.float64
    )
    h = x @ np.asarray(W1, np.float64) + np.asarray(b1, np.float64)
    erf = np.frompyfunc(math.erf, 1, 1)
    g = h * 0.5 * (1.0 + erf(h / math.sqrt(2.0)).astype(np.float64))
    r = g @ np.asarray(W2, np.float64) + np.asarray(b2, np.float64)
    return r.astype(np.float32)


def kernel(
    appearance_tokens,
    target_features,
    noise,
    mask_token,
    ln_w,
    ln_b,
    W1,
    b1,
    W2,
    b2,
):
    from concourse.bass_utils import run_bass_kernel_spmd

    nc = _get_nc()

    r = _host_r(mask_token, ln_w, ln_b, W1, b1, W2, b2)
    in_maps = [
        {"tf": tfull_i, **_const_inputs(r)} for tfull_i in _shard_tf(target_features)
    ]

    res = run_bass_kernel_spmd(nc, in_maps, list(range(NCORES)))
    total = 0.0
    for i in range(NCORES):
        total += float(np.asarray(res.results[i]["out"], np.float64).sum())

    loss = 2.0 * total / C / (256.0 + 1e-8)
    return np.float32(loss)


def _const_inputs(r):
    """Constant device inputs derived from the decoder vector r."""
    negr = np.ascontiguousarray(-r.reshape(1, C), np.float32)
    ones16 = np.ones((1, BPC), np.float32)
    # emat[:, b*16+m] = 1/N if m == b else 0 — one-hot columns scaled so the
    # partition-reduction matmul lands mean_n directly in PSUM row b.
    emat = np.zeros((PPB, BPC * BPC), np.float32)
    for b in range(BPC):
        emat[:, b * BPC + b] = 1.0 / N
    return {"negr": negr, "ones16": ones16, "emat": emat}


def _shard_tf(target_features):
    return np.ascontiguousarray(target_features, np.float32).reshape(
        NCORES, BPC, PPB, FREE
    )


# revision 32
# speedup vs baseline: 1.1078x; 1.0342x over previous
"""Trainium2 distributed kernel for the AppearanceReconstruction loss.

Math note (exact identity, not an approximation): the MAE shuffle/gather in
the reference collapses — restored[b,p] is appearance_tokens[b,p] on kept
slots (which the mask multiplies by zero) and mask_token on masked slots.
Every row has exactly num_masked = 2 masked slots, and the decoder output at
a masked slot is the same single vector r = MLP(LN(mask_token)) for all
(b,p). Hence

    loss = 2 * sum_b mean_c((r_c - pooled[b,c])^2) / (256 + 1e-8)
    pooled[b] = mean_n target_features[b,n,:]

The memory-bound part (reading all of target_features, 402 MB) runs on the 8
NeuronCores, data-parallel over B (16 rows per core). Each core streams its
50 MB shard through SBUF in 3 MB tiles and reduces over N with TensorEngine
matmuls against one-hot columns (scaled by 1/N), accumulating the 16 row
means into a [16, 768] PSUM tile. A short vector-engine epilogue computes
sum_c (pooled - r)^2 per row; the host sums the 8x16 partials.
"""

import math

import numpy as np

B, N, C = 128, 1024, 768
NCORES = 8
BPC = B // NCORES  # rows per core
PPB = 128  # SBUF partitions per row-tile
NSUB = N // PPB  # n-rows folded into each partition's free dim
FREE = NSUB * C  # 6144 floats per partition per row-tile
LN_EPS = 1e-5

_CACHE = {}

# kernel structure knobs (A/B-tested on hardware; defaults = best measured)
_VARIANT = {
    "last_split": True,
    "out_ring": "scalar",
    "ring_alt": False,
    "head_split": False,
    "bufs": 6,
}


def _build():
    import concourse.bass as bass  # noqa: F401
    import concourse.tile as tile
    from concourse import bacc, mybir

    f32 = mybir.dt.float32
    f32r = mybir.dt.float32r
    AL = mybir.AluOpType
    AX = mybir.AxisListType

    nc = bacc.Bacc(
        "TRN2", target_bir_lowering=False, debug=False, num_devices=NCORES
    )
    tf = nc.dram_tensor("tf", [BPC, PPB, FREE], f32r, kind="ExternalInput")
    negr = nc.dram_tensor("negr", [1, C], f32r, kind="ExternalInput")
    ones16 = nc.dram_tensor("ones16", [1, BPC], f32r, kind="ExternalInput")
    emat = nc.dram_tensor("emat", [PPB, BPC * BPC], f32r, kind="ExternalInput")
    out = nc.dram_tensor("out", [BPC, 1], f32, kind="ExternalOutput")

    with tile.TileContext(nc) as tc:
        with (
            tc.tile_pool(name="consts", bufs=1) as cpool,
            tc.tile_pool(name="data", bufs=_VARIANT["bufs"]) as dpool,
            tc.tile_pool(name="epi", bufs=1) as epool,
            tc.tile_pool(name="psum", bufs=1, space="PSUM") as ppool,
        ):
            # issue the first data DMAs before the tiny const loads so the
            # big stream starts as early as possible; with head_split the
            # first row goes as two halves issued from BOTH HWDGE rings so
            # their descriptor generation runs in parallel at t=0
            hhalf = NSUB // 2
            if _VARIANT["head_split"]:
                t0_tile = dpool.tile([PPB, hhalf * C], f32r, tag="data")
                nc.sync.dma_start(out=t0_tile[:], in_=tf.ap()[0, :, 0 : hhalf * C])
                t0b_tile = dpool.tile([PPB, hhalf * C], f32r, tag="data")
                nc.scalar.dma_start(
                    out=t0b_tile[:], in_=tf.ap()[0, :, hhalf * C : FREE]
                )
            else:
                t0_tile = dpool.tile([PPB, FREE], f32r, tag="data")
                nc.sync.dma_start(out=t0_tile[:], in_=tf.ap()[0])
                t0b_tile = None

            # const loads go on the ACT HWDGE ring so the SP ring's first job
            # is the 3 MB stream itself
            emat_sb = cpool.tile([PPB, BPC * BPC], f32r)
            nc.scalar.dma_start(out=emat_sb[:], in_=emat.ap())
            negr_sb = cpool.tile([1, C], f32r)
            nc.scalar.dma_start(out=negr_sb[:], in_=negr.ap())
            ones16_sb = cpool.tile([1, BPC], f32r)
            nc.scalar.dma_start(out=ones16_sb[:], in_=ones16.ap())

            # single [16, 768] accumulator spanning two PSUM banks; each
            # matmul's out AP stays within one bank (512 | 256)
            ps = ppool.tile([BPC, C], f32)

            # (row, sub_lo, sub_hi) chunks; full 3 MB rows keep the DMA
            # stream at peak rate, only the last row is halved so the
            # post-final-DMA PE tail is half a row
            half = NSUB // 2
            quart = NSUB // 4
            if _VARIANT["head_split"]:
                chunks = [(0, 0, half), (0, half, NSUB)]
            else:
                chunks = [(0, 0, NSUB)]
            chunks += [(b, 0, NSUB) for b in range(1, BPC - 1)]
            if _VARIANT["last_split"]:
                chunks += [
                    (BPC - 1, 0, half),
                    (BPC - 1, half, half + quart),
                    (BPC - 1, half + quart, NSUB),
                ]
            else:
                chunks += [(BPC - 1, 0, NSUB)]

            for ci, (b, lo, hi) in enumerate(chunks):
                if ci == 0:
                    t = t0_tile
                elif ci == 1 and t0b_tile is not None:
                    t = t0b_tile
                else:
                    t = dpool.tile([PPB, (hi - lo) * C], f32r, tag="data")
                    # optionally alternate the two HWDGE rings (SP/ACT) so
                    # descriptor generation of consecutive transfers overlaps;
                    # the final two chunks stay on one ring to preserve their
                    # completion order (the PE tail depends on the last chunk
                    # alone finishing last)
                    if _VARIANT["ring_alt"] and ci < len(chunks) - 2:
                        eng = nc.sync if ci % 2 == 0 else nc.scalar
                    else:
                        eng = nc.sync
                    eng.dma_start(
                        out=t[:], in_=tf.ap()[b, :, lo * C : hi * C]
                    )
                # float32r: same 4-byte layout, 4x faster PE streaming; the
                # reduced-precision multiply is far inside the loss tolerance.
                lhsT = emat_sb[:, b * BPC : (b + 1) * BPC]
                first = ci == 0
                last = ci == len(chunks) - 1
                for sub in range(lo, hi):
                    nc.tensor.matmul(
                        ps[:, 0:512],
                        lhsT,
                        t[:, (sub - lo) * C : (sub - lo) * C + 512],
                        start=first and sub == lo,
                        stop=last and sub == hi - 1,
                    )
                for sub in range(lo, hi):
                    nc.tensor.matmul(
                        ps[:, 512:768],
                        lhsT,
                        t[:, (sub - lo) * C + 512 : (sub - lo + 1) * C],
                        start=first and sub == lo,
                        stop=last and sub == hi - 1,
                    )
                if ci == 0:
                    # fold the "- r" into the accumulation: one K=1 matmul
                    # adds -r_c to every row, early so it is off the tail.
                    # PSUM then holds (pooled_mean - r) directly and the
                    # epilogue shrinks to square + reduce.
                    nc.tensor.matmul(
                        ps[:, 0:512],
                        ones16_sb[:],
                        negr_sb[:, 0:512],
                        start=False,
                        stop=False,
                    )
                    nc.tensor.matmul(
                        ps[:, 512:768],
                        ones16_sb[:],
                        negr_sb[:, 512:768],
                        start=False,
                        stop=False,
                    )

            # one ACT instruction: square every element of (pooled - r) and
            # row-sum into s — single PSUM read, runs on the idle ACT engine
            sq = epool.tile([BPC, C], f32)
            s = epool.tile([BPC, 1], f32)
            nc.scalar.activation(
                out=sq[:],
                in_=ps[:],
                func=mybir.ActivationFunctionType.Square,
                accum_out=s[:],
            )
            # output DMA on the ACT HWDGE ring so it never queues behind the
            # SP ring's bulk data stream
            out_eng = nc.scalar if _VARIANT["out_ring"] == "scalar" else nc.sync
            out_eng.dma_start(out=out.ap(), in_=s[:])

    nc.compile()
    return nc


def _get_nc():
    nc = _CACHE.get("nc")
    if nc is None:
        nc = _build()
        _CACHE["nc"] = nc
    return nc


def _host_r(mask_token, ln_w, ln_b, W1, b1, W2, b2):
    """r = Linear2(gelu_exact(Linear1(LayerNorm(mask_token)))) — one 768-vec."""
    mt = np.asarray(mask_token, np.float64).reshape(C)
    mu = mt.mean()
    var = ((mt - mu) ** 2).mean()
    x = (mt - mu) / np.sqrt(var + LN_EPS) * np.asarray(ln_w, np.float64) + np.asarray(
        ln_b, np.float64
    )
    h = x @ np.asarray(W1, np.float64) + np.asarray(b1, np.float64)
    erf = np.frompyfunc(math.erf, 1, 1)
    g = h * 0.5 * (1.0 + erf(h / math.sqrt(2.0)).astype(np.float64))
    r = g @ np.asarray(W2, np.float64) + np.asarray(b2, np.float64)
    return r.astype(np.float32)


def kernel(
    appearance_tokens,
    target_features,
    noise,
    mask_token,
    ln_w,
    ln_b,
    W1,
    b1,
    W2,
    b2,
):
    from concourse.bass_utils import run_bass_kernel_spmd

    nc = _get_nc()

    r = _host_r(mask_token, ln_w, ln_b, W1, b1, W2, b2)
    in_maps = [
        {"tf": tfull_i, **_const_inputs(r)} for tfull_i in _shard_tf(target_features)
    ]

    res = run_bass_kernel_spmd(nc, in_maps, list(range(NCORES)))
    total = 0.0
    for i in range(NCORES):
        total += float(np.asarray(res.results[i]["out"], np.float64).sum())

    loss = 2.0 * total / C / (256.0 + 1e-8)
    return np.float32(loss)


def _const_inputs(r):
    """Constant device inputs derived from the decoder vector r."""
    negr = np.ascontiguousarray(-r.reshape(1, C), np.float32)
    ones16 = np.ones((1, BPC), np.float32)
    # emat[:, b*16+m] = 1/N if m == b else 0 — one-hot columns scaled so the
    # partition-reduction matmul lands mean_n directly in PSUM row b.
    emat = np.zeros((PPB, BPC * BPC), np.float32)
    for b in range(BPC):
        emat[:, b * BPC + b] = 1.0 / N
    return {"negr": negr, "ones16": ones16, "emat": emat}


def _shard_tf(target_features):
    return np.ascontiguousarray(target_features, np.float32).reshape(
        NCORES, BPC, PPB, FREE
    )
